# revision 3
# baseline (speedup 1.0000x reference)
"""LIF spiking-neuron recurrence (nn_LIFSpike) as a Bass/Tile kernel on 8
Trainium2 NeuronCores — v2: engine-balanced, nibble-packed output.

Math (reference): thre = tanh(w); over T=4 steps along the last axis,
    u_t = TAU * u_{t-1} * (1 - o_{t-1}) + x_t
    o_t = (u_t > thre)            # heaviside, output per step
with u_0 = o_0 = 0 and TAU = 0.25.

Rescaled recurrence (bit-exact): w_t = 4^t * u_t. Since 4*TAU = 1,
    w_t = w_{t-1} * [w_{t-1} <= thre_{t-1}] + x~_t,  x~_t = 4^t * x_t,
    o_t = [w_t > thre_t],         thre_t = 4^t * thre.
Powers of two commute with fp32 rounding exactly (no overflow here), so
each step is ONE fp32 add rounding — identical bits to the reference.
The leak constant disappears: every recurrence matmul uses the identity.

Engine split per tile (npp neurons/partition, planes time-planar):
  ACT   o_t = sigmoid(2^40*(w_t - thre_t)) -> SBUF f32 {0,1} (saturated
        LUT; only w == thre exactly gives 0.5 — handled below)  [4 ops]
  DVE   m_t = (o_t is_lt 1.0) mult w_t     -> SBUF              [3 ops]
        (is_lt 1.0 treats the 0.5 boundary case exactly like the
        reference's (1-o) reset; reads w from PSUM + o from SBUF)
  PE    w_{t+1} = I@m_t (start) + I@x~_{t+1} (accum) -> PSUM    [4 MM]
        pack: pk = sum_t diag(2^t) @ o_t   -> PSUM              [8 MM]
  DVE   nib = u8(pk)                        -> SBUF             [1 op]
Output is 1 byte per neuron (4 spike bits in a nibble) instead of 4:
store traffic drops 4x (4 MiB -> 1 MiB per core). Host unpacks bits.

DMA: loads ride the SP HWDGE ring (all triggers issued up front; x tiles
are fully resident in SBUF so no buffer back-pressure ever stalls the
ring). Stores ride the ACT ring, emitted after all sigmoids in ACT
program order so a store's semaphore wait never delays compute. The
first tile loads plane-by-plane so compute starts after ~0.5 MiB.

Sharding: pure elementwise -> batch dim split into 8 chunks of
[8,128,32,32,4], one per core, no communication.
"""

import numpy as np

TAU = 0.25
T = 4
N_CORES = 8
PART = 128
import os as _os

if _os.environ.get("LIF_TILES"):
    TILE_NPP = tuple(int(v) for v in _os.environ["LIF_TILES"].split(","))
else:
    TILE_NPP = (1024,) * 8  # neurons/partition per tile
NPP_TOTAL = sum(TILE_NPP)  # 8192
FULL_SHAPE = (64, 128, 32, 32, T)
CORE_ELEMS = PART * NPP_TOTAL * T  # 4,194,304 (input f32 elems)
OUT_ELEMS = PART * NPP_TOTAL  # 1,048,576 (output u8 nibbles)

SIGMOID_K = float(2.0**40)
MMF = 512  # fp32 matmul free-dim / PSUM bank width

_cache: dict = {}


def _wdiag() -> np.ndarray:
    """[128, 4*128] f32: columns [t*128:(t+1)*128] = diag(2^t)."""
    wd = np.zeros((PART, 4 * PART), np.float32)
    for t in range(4):
        wd[:, t * PART : (t + 1) * PART][np.arange(PART), np.arange(PART)] = (
            2.0**t
        )
    return wd


def _build(thre: float):
    import concourse.bacc as bacc
    import concourse.mybir as mybir
    from concourse import tile

    f32 = mybir.dt.float32
    u8 = mybir.dt.uint8
    Alu = mybir.AluOpType
    Act = mybir.ActivationFunctionType

    nc = bacc.Bacc("TRN2", target_bir_lowering=False, debug=False)
    xd = nc.dram_tensor("x", [CORE_ELEMS], f32, kind="ExternalInput").ap()
    wdd = nc.dram_tensor("wd", [PART, 4 * PART], f32, kind="ExternalInput").ap()
    od = nc.dram_tensor("o", [OUT_ELEMS], u8, kind="ExternalOutput").ap()

    with tile.TileContext(nc) as tc:
        with (
            tc.tile_pool(name="const", bufs=1) as cpool,
            tc.tile_pool(name="xp", bufs=len(TILE_NPP)) as xpool,
            tc.tile_pool(name="op", bufs=2) as opool,
            tc.tile_pool(name="mp", bufs=3) as mpool,
            tc.tile_pool(name="nib", bufs=len(TILE_NPP)) as nibpool,
            tc.psum_pool(name="wps", bufs=2) as wpool,
            tc.psum_pool(name="pps", bufs=2) as ppool,
        ):
            # constants: diag weights (ACT ring so the load ring stays
            # clear for x) and the 4 per-step sigmoid biases
            wd = cpool.tile([PART, 4 * PART], f32, tag="wd")
            nc.scalar.dma_start(wd[:], wdd)
            ident = wd[:, 0:PART]
            biases = []
            for t in range(T):
                b = cpool.tile([PART, 1], f32, tag=f"b{t}")
                nc.vector.memset(b[:], float(-SIGMOID_K * thre * (4.0**t)))
                biases.append(b)

            stores = []  # (osrc, nib) deferred to end of ACT program
            base = 0
            obase = 0
            for i, npp in enumerate(TILE_NPP):
                free = npp * T
                xsrc = xd[base : base + PART * free].rearrange(
                    "(p f) -> p f", f=free
                )
                osrc = od[obase : obase + PART * npp].rearrange(
                    "(p f) -> p f", f=npp
                )
                base += PART * free
                obase += PART * npp

                xt = xpool.tile([PART, free], f32, tag="x")
                if i == 0:
                    # plane-granular first load: compute starts after the
                    # first plane lands instead of the whole tile
                    for t in range(T):
                        nc.sync.dma_start(
                            xt[:, t * npp : (t + 1) * npp],
                            xsrc[:, t * npp : (t + 1) * npp],
                        )
                else:
                    nc.sync.dma_start(xt[:], xsrc)

                ot = opool.tile([PART, free], f32, tag="o")
                w_ap = xt[:, 0:npp]  # w_0 = x~_0
                for t in range(T):
                    nc.scalar.activation(
                        ot[:, t * npp : (t + 1) * npp],
                        w_ap,
                        Act.Sigmoid,
                        bias=biases[t][:],
                        scale=SIGMOID_K,
                    )
                    if t < T - 1:
                        m = mpool.tile([PART, npp], f32, tag="m")
                        nc.vector.scalar_tensor_tensor(
                            m[:],
                            ot[:, t * npp : (t + 1) * npp],
                            1.0,
                            w_ap,
                            Alu.is_lt,
                            Alu.mult,
                        )
                        wn = wpool.tile([PART, npp], f32, tag="w")
                        for h in range(0, npp, MMF):
                            sl = slice(h, h + MMF)
                            nc.tensor.matmul(
                                wn[:, sl],
                                ident,
                                m[:, sl],
                                start=True,
                                stop=False,
                            )
                            nc.tensor.matmul(
                                wn[:, sl],
                                ident,
                                xt[:, (t + 1) * npp + h : (t + 1) * npp + h + MMF],
                                start=False,
                                stop=True,
                            )
                        w_ap = wn[:]
                # pack the 4 spike planes into a nibble per neuron
                pk = ppool.tile([PART, npp], f32, tag="pk")
                for h in range(0, npp, MMF):
                    sl = slice(h, h + MMF)
                    for t in range(T):
                        nc.tensor.matmul(
                            pk[:, sl],
                            wd[:, t * PART : (t + 1) * PART],
                            ot[:, t * npp + h : t * npp + h + MMF],
                            start=(t == 0),
                            stop=(t == T - 1),
                        )
                nib = nibpool.tile([PART, npp], u8, tag="nib")
                nc.vector.tensor_copy(nib[:], pk[:])
                stores.append((osrc, nib))

            # stores last in ACT program order: their semaphore waits can
            # never delay a sigmoid
            for osrc, nib in stores:
                nc.scalar.dma_start(osrc, nib[:])
    nc.compile()
    return nc


def _get_nc(thre: float):
    key = round(thre, 9)
    if key not in _cache:
        _cache[key] = _build(thre)
    return _cache[key]


def _shard(x: np.ndarray) -> np.ndarray:
    """[64,128,32,32,4] f32 -> [N_CORES, CORE_ELEMS] time-planar per tile,
    plane t scaled by 4^t (exact)."""
    xc = x.reshape(N_CORES, PART, NPP_TOTAL, T)  # [core, part, neuron, t]
    scale = (4.0 ** np.arange(T)).astype(np.float32)  # [T]
    out = np.empty((N_CORES, CORE_ELEMS), np.float32)
    base = 0
    npp_base = 0
    for npp in TILE_NPP:
        blk = xc[:, :, npp_base : npp_base + npp, :]  # [C, P, npp, T]
        blk = blk.transpose(0, 1, 3, 2) * scale[None, None, :, None]
        n = PART * T * npp
        out[:, base : base + n] = blk.reshape(N_CORES, n)
        base += n
        npp_base += npp
    return out


def _unshard(out_nib: np.ndarray) -> np.ndarray:
    """[N_CORES, OUT_ELEMS] uint8 nibbles -> full-shape f32 spikes."""
    res = np.empty((N_CORES, PART, NPP_TOTAL, T), np.float32)
    base = 0
    npp_base = 0
    for npp in TILE_NPP:
        n = PART * npp
        blk = out_nib[:, base : base + n].reshape(N_CORES, PART, npp)
        for t in range(T):
            res[:, :, npp_base : npp_base + npp, t] = (blk >> t) & 1
        base += n
        npp_base += npp
    return res.reshape(FULL_SHAPE)


def _run(x_planar, thre: float, **run_kwargs):
    from concourse.bass_utils import run_bass_kernel_spmd

    nc = _get_nc(thre)
    wd = _wdiag()
    in_maps = [
        {"x": np.ascontiguousarray(x_planar[c]), "wd": wd}
        for c in range(N_CORES)
    ]
    return run_bass_kernel_spmd(
        nc, in_maps, core_ids=list(range(N_CORES)), **run_kwargs
    )


def kernel(x, w):
    x = np.asarray(x, dtype=np.float32)
    assert x.shape == FULL_SHAPE, x.shape
    thre = float(np.tanh(np.float32(np.asarray(w))))
    xs = _shard(x)
    r = _run(xs, thre)
    out = np.stack([np.asarray(r.results[c]["o"]) for c in range(N_CORES)])
    return _unshard(out)


# revision 10
# speedup vs baseline: 2.0853x; 2.0853x over previous
"""LIF spiking-neuron recurrence (nn_LIFSpike) as a Bass/Tile kernel on 8
Trainium2 NeuronCores.

Math (reference): thre = tanh(w); over T=4 steps along the last axis,
    u_t = TAU * u_{t-1} * (1 - o_{t-1}) + x_t
    o_t = (u_t > thre)            # heaviside, output per step
with u_0 = o_0 = 0 and TAU = 0.25.

Bit-exactness: TAU = 0.25 is a power of two and (1 - o) in {0,1}, so
    u_t = fl(0.25 * (u * [u <= thre]) + x_t)
matches the reference exactly. Per step this is two fused DVE
scalar_tensor_tensor ops:
    m  = (u is_le thre) mult u         # reset: u or 0, exact
    u' = (m mult 0.25) add x_t         # leak + input, single rounding
The spike output runs on the Scalar engine as a saturated sigmoid
    o = sigmoid(2^40 * (u - thre))
whose LUT saturates to exactly 0.0 / 1.0 (verified bit-exact vs the
reference on hardware), written directly as uint8 (spikes are {0,1}), which
quarters the output DMA traffic; the host unshard converts back to f32.
Only |u - thre| < ~3e-8 could round the wrong way (~1 element in 3e7,
single-output perturbation only — the recurrence itself uses the exact
is_le comparison).

Sharding/layout: pure elementwise per-neuron -> split the batch dim into 8
chunks of [8,128,32,32,4] (4,194,304 contiguous f32), one per core, no
communication. During host-side sharding each tile's [neurons x 4 steps]
block is transposed to time-planar [4 x neurons] so every device-side access
is unit-stride (interleaved stride-4 access costs ~1.9x on both DVE and
ACT); the output is transposed back during unshard. Tiles are sized
[1024,1024,2048,2048,1024,1024] neurons/partition: small edge tiles shorten
the first-load latency and the final store tail, large middle tiles keep
per-op overhead low. The first tile additionally loads plane-by-plane so
the DVE chain starts after ~0.5 MiB instead of 2 MiB. Loads ride the SP
HWDGE ring; stores ride the ACT HWDGE ring (a store issues right after the
sigmoid that produced its tile, so it never delays a load behind it in a
shared FIFO).
"""

import numpy as np

TAU = 0.25
T = 4
N_CORES = 8
PART = 128
import os as _os

if _os.environ.get("LIF_TILES"):
    TILE_NPP = tuple(int(v) for v in _os.environ["LIF_TILES"].split(","))
else:
    # small last tile shortens the final compute+store tail
    TILE_NPP = (1024, 1024, 1024, 1024, 1024, 1280, 1280, 512)
NPP_TOTAL = sum(TILE_NPP)  # 8192
FULL_SHAPE = (64, 128, 32, 32, T)
CORE_ELEMS = PART * NPP_TOTAL * T  # 4,194,304

SIGMOID_K = float(2.0**40)

_cache: dict = {}


def _build(thre: float, variant: str):
    import concourse.bacc as bacc
    import concourse.mybir as mybir
    from concourse import tile

    f32 = mybir.dt.float32
    u8 = mybir.dt.uint8
    Alu = mybir.AluOpType
    Act = mybir.ActivationFunctionType

    nc = bacc.Bacc("TRN2", target_bir_lowering=False, debug=False)
    # flat per-core DRAM layout: tiles back-to-back, tile i is
    # [PART, T, NPP_i] C-order (partition line = T*NPP_i contiguous f32)
    xd = nc.dram_tensor("x", [CORE_ELEMS], f32, kind="ExternalInput").ap()
    od = nc.dram_tensor("o", [CORE_ELEMS], u8, kind="ExternalOutput").ap()

    with tile.TileContext(nc) as tc:
        with (
            tc.tile_pool(name="const", bufs=1) as cpool,
            tc.tile_pool(name="xp", bufs=5) as xpool,
            tc.tile_pool(name="op", bufs=2) as opool,
            tc.tile_pool(name="work", bufs=3) as work,
        ):
            # w_t = 4^t * u_t rescaling (host pre-scales plane t by 4^t,
            # exact): the leak+add becomes a plain TT add (GPSIMD-capable)
            # and the per-step threshold is thre_t = 4^t * thre.
            biases = []
            for t in range(T):
                b = cpool.tile([PART, 1], f32, tag=f"b{t}")
                nc.vector.memset(
                    b[:], float(-SIGMOID_K * (thre * (4.0**t)))
                )
                biases.append(b)

            base = 0
            for i, npp in enumerate(TILE_NPP):
                free = npp * T
                xsrc = xd[base : base + PART * free].rearrange(
                    "(p f) -> p f", f=free
                )
                osrc = od[base : base + PART * free].rearrange(
                    "(p f) -> p f", f=free
                )
                base += PART * free

                xt = xpool.tile([PART, free], f32, tag="x")
                if i == 0:
                    # plane-granular first load: compute starts after the
                    # first plane lands instead of the whole tile
                    for t in range(T):
                        nc.sync.dma_start(
                            xt[:, t * npp : (t + 1) * npp],
                            xsrc[:, t * npp : (t + 1) * npp],
                        )
                else:
                    nc.sync.dma_start(xt[:], xsrc)
                ot = opool.tile([PART, free], u8, tag="o")

                u = xt[:, 0:npp]  # w_0 = x~_0 (u_0 = o_0 = 0)
                for t in range(T):
                    nc.scalar.activation(
                        ot[:, t * npp : (t + 1) * npp],
                        u,
                        Act.Sigmoid,
                        bias=biases[t][:],
                        scale=SIGMOID_K,
                    )
                    if t < T - 1:
                        m = work.tile([PART, npp], f32, tag="m")
                        nc.vector.scalar_tensor_tensor(
                            m[:], u, float(thre * (4.0**t)), u, Alu.is_le, Alu.mult
                        )
                        un = work.tile([PART, npp], f32, tag="u")
                        # offload one add per step-chain to the idle GPSIMD
                        # engine: DVE drops from 6 to 5 ops/tile and keeps
                        # pace with the load stream (no compute drain)
                        eng = nc.gpsimd if t == 1 else nc.vector
                        eng.tensor_tensor(
                            un[:],
                            m[:],
                            xt[:, (t + 1) * npp : (t + 2) * npp],
                            Alu.add,
                        )
                        u = un[:]
                # store on the ACT HWDGE ring: issues right after this tile's
                # last sigmoid in ACT program order, never blocking SP loads
                nc.scalar.dma_start(osrc, ot[:])
    nc.compile()
    return nc


def _get_nc(thre: float, variant: str):
    key = (round(thre, 9), variant)
    if key not in _cache:
        _cache[key] = _build(thre, variant)
    return _cache[key]


def _shard(x: np.ndarray) -> np.ndarray:
    """[64,128,32,32,4] f32 -> [N_CORES, CORE_ELEMS] time-planar per tile."""
    xc = x.reshape(N_CORES, PART, NPP_TOTAL, T)  # [core, part, neuron, t]
    scale = (4.0 ** np.arange(T)).astype(np.float32)
    out = np.empty((N_CORES, CORE_ELEMS), np.float32)
    base = 0
    npp_base = 0
    for npp in TILE_NPP:
        blk = xc[:, :, npp_base : npp_base + npp, :]  # [C, P, npp, T]
        # plane t scaled by 4^t (exact power-of-two, commutes with fp32
        # rounding): w_t = 4^t*u_t recurrence needs no TAU multiply
        blk = blk.transpose(0, 1, 3, 2) * scale[None, None, :, None]
        n = PART * T * npp
        out[:, base : base + n] = blk.reshape(N_CORES, n)
        base += n
        npp_base += npp
    return out


def _unshard(out_planar: np.ndarray) -> np.ndarray:
    """[N_CORES, CORE_ELEMS] uint8 time-planar -> full-shape f32."""
    res = np.empty((N_CORES, PART, NPP_TOTAL, T), np.uint8)
    base = 0
    npp_base = 0
    for npp in TILE_NPP:
        n = PART * T * npp
        blk = out_planar[:, base : base + n].reshape(N_CORES, PART, T, npp)
        res[:, :, npp_base : npp_base + npp, :] = blk.transpose(0, 1, 3, 2)
        base += n
        npp_base += npp
    return res.reshape(FULL_SHAPE).astype(np.float32)


def _run(x_planar, thre: float, variant: str = "sigmoid", **run_kwargs):
    from concourse.bass_utils import run_bass_kernel_spmd

    nc = _get_nc(thre, variant)
    in_maps = [{"x": np.ascontiguousarray(x_planar[c])} for c in range(N_CORES)]
    return run_bass_kernel_spmd(
        nc, in_maps, core_ids=list(range(N_CORES)), **run_kwargs
    )


def kernel(x, w):
    x = np.asarray(x, dtype=np.float32)
    assert x.shape == FULL_SHAPE, x.shape
    thre = float(np.tanh(np.float32(np.asarray(w))))
    xs = _shard(x)
    r = _run(xs, thre)
    out = np.stack([np.asarray(r.results[c]["o"]) for c in range(N_CORES)])
    return _unshard(out)

